# revision 30
# baseline (speedup 1.0000x reference)
"""CCNF RK4 sampling kernel for 8 Trainium2 NeuronCores — v2.

Data-parallel across cores (2048 -> 256/core), and each core's batch is
split into TWO groups of 128 samples whose serial RK4 chains are
software-pipelined half-an-eval apart, so one group's L1 sigmoid/GLU
phase overlaps the other group's L2/L3 phase on the ACT/DVE engines.

The v1 kernel was latency-bound on the per-eval serial chain
(theta-MM -> 4x(sigma,GLU) -> L2 -> 4x(sigma,GLU) -> L3 -> RK4-STT ->
theta-MM', ~6.9us/eval).  v2 shortens the chain per group and hides the
rest with the second group:

  - whole-bank ops: sigma is ONE activation op per layer over a full
    [128, 4, 128] PSUM bank (4 chunks), GLU is ONE STT.  Bias made
    unnecessary: the time row t*W1[32] + b1 ride the theta-stationary
    ([34, 128]: theta rows + t row + ones row, maintained by gpsimd
    memsets on the idle Pool engine).
  - F-shortcut: tx = theta_s + c*k feeds L1 only through W1theta, so
    L1pre(e+1) = [ctx + theta_s + t] (pre-issued off-chain) +
    h2_e @ Fc where F = W3 @ W1[0:32] is precomputed host-side and
    applied as fp8 DoubleRow matmuls.  This removes L3->STT->theta-MM
    (two sem hops + a DVE op) from 3 of 4 eval boundaries.
  - RK4 combination in PSUM: acc += w_e * k_e via duplicate cheap L3
    DR matmuls with pre-scaled W3 variants; one STT per STEP updates
    the f32 theta state (thF), one ACT copy refreshes the bf16
    matmul-input copy.  (v1 spent 2 DVE STTs per eval here.)
  - fp8 scales: h2 is written scaled by s_h=1/4 (free in the GLU STT
    scalar) so Fc = F*c/s_h and w3 variants stay in fp8 normal range.

Numpy-probed accuracy of this exact quantization pipeline: 1.13e-2
(gate 2e-2).  Cost model: ~3.8us per eval-pair vs v1's 6.9us per eval.
"""

import os

import numpy as np
from ml_dtypes import bfloat16 as _bf16
from ml_dtypes import float8_e4m3 as _f8np

N_CORES = 8
G = 2          # pipelined sample groups per core
CTX8 = bool(int(os.environ.get("KERNEL_CTX8", "0")))  # fp8 DoubleRow ctx matmuls
SPLIT = bool(int(os.environ.get("KERNEL_SPLIT", "1")))  # pair-split sigma/GLU ops
OP_LABELS = {}  # instruction name -> human label (for the trace analyzer)
S_H = 0.25     # h2 scale carried in the GLU2 STT scalar


def _build_program(theta0, context, W1, b1, W2, b2, W3, b3, n_steps):
    import concourse.mybir as mybir
    import concourse.tile as tile
    from concourse import bacc

    f32 = mybir.dt.float32
    f32r = mybir.dt.float32r
    bf16 = mybir.dt.bfloat16
    f8 = mybir.dt.float8e4
    DR = mybir.MatmulPerfMode.DoubleRow
    ALU = mybir.AluOpType
    SIGMOID = mybir.ActivationFunctionType.Sigmoid

    B, D = theta0.shape          # 2048, 32
    C = context.shape[1]         # 128
    IN, H2 = W1.shape            # 161, 1024
    H = W2.shape[0]              # 512
    assert H2 == 2 * H and W2.shape[1] == 2 * H and W3.shape == (H, D)
    assert IN == D + 1 + C
    assert B % (N_CORES * G) == 0
    Bs = B // N_CORES            # 256
    Ng = Bs // G                 # 128
    steps = int(n_steps)
    dt = 1.0 / steps
    MJ = H // 128                # 4 column chunks per GLU half
    KCP = MJ // 2                # 2 DoubleRow pairs over the H contraction
    NEV = 4 * steps
    DT2 = D + 2                  # moving rows: theta(32) + t(1) + ones(1)

    b2f = np.asarray(b2, np.float32)
    b3f = np.asarray(b3, np.float32)
    b2nz = bool(np.any(b2f))
    b3nz = bool(np.any(b3f))

    # t value per eval (t = idx * dt/2)
    IOFF = (0, 1, 1, 2)
    TVAL = [(2 * (e // 4) + IOFF[e % 4]) * (dt / 2.0) for e in range(NEV + 1)]

    # ---- host-side layout prep (shared across cores) ----
    W1f = np.asarray(W1, np.float32)
    w1c_h = np.ascontiguousarray(W1f[D + 1:])                    # [128, 1024]
    if CTX8:
        # DoubleRow pairing along the 128 ctx rows: k = plane*64 + p
        w1c8_h = np.ascontiguousarray(
            w1c_h.reshape(2, C // 2, 2 * H).transpose(1, 0, 2))  # [64, 2, 1024]
    w1tb_h = np.concatenate(
        [W1f[0:D + 1], np.asarray(b1, np.float32).reshape(1, 2 * H)], axis=0
    )                                                            # [34, 1024]
    w2_h = np.ascontiguousarray(
        np.asarray(W2, np.float32)
        .reshape(KCP, 2, 128, 2 * H).transpose(2, 0, 1, 3)
        .reshape(128, KCP * 2 * 2 * H)
    )

    W3f = np.asarray(W3, np.float32)

    def drpack(w, ncol):  # [H, ncol] -> [128, KCP, 2, ncol]
        return w.reshape(KCP, 2, 128, ncol).transpose(2, 0, 1, 3)

    # w3 variants scaled by w_e / s_h (w_e in {1, 2})
    w3v_h = np.ascontiguousarray(np.stack(
        [drpack(W3f * (1.0 / S_H), D), drpack(W3f * (2.0 / S_H), D)], axis=3
    ).reshape(128, KCP * 2 * 2 * D))                             # [128, P, pl, v, D]
    # F variants scaled by c_e / s_h (c_e in {dt/2, dt})
    F_h = W3f @ W1f[0:D]                                         # [512, 1024]
    fv_h = np.ascontiguousarray(np.stack(
        [drpack(F_h * (0.5 * dt / S_H), 2 * H), drpack(F_h * (dt / S_H), 2 * H)],
        axis=3,
    ).reshape(128, KCP * 2 * 2 * 2 * H))                         # [128, P, pl, v, 1024]

    # ---- build the bass program (same program on all 8 cores) ----
    nc = bacc.Bacc("TRN2", target_bir_lowering=False)

    if CTX8:
        d_ctxw = nc.dram_tensor("ctxw", [C // 2, 2 * (G * Ng + 2 * H)], f8,
                                kind="ExternalInput")
    else:
        d_ctxw = nc.dram_tensor("ctxw", [C, G * Ng + 2 * H], bf16,
                                kind="ExternalInput")
    d_thw = nc.dram_tensor("thw", [DT2, G * Ng + 2 * H], bf16, kind="ExternalInput")
    d_thF = nc.dram_tensor("thF", [D, Bs], f32, kind="ExternalInput")
    d_w2 = nc.dram_tensor("w2", [128, KCP * 2 * 2 * H], f8, kind="ExternalInput")
    d_w3v = nc.dram_tensor("w3v", [128, KCP * 2 * 2 * D], f8, kind="ExternalInput")
    d_fv = nc.dram_tensor("fv", [128, KCP * 2 * 2 * 2 * H], f8, kind="ExternalInput")
    # bias fallbacks (all-zero in the reference problem): bias values ride as
    # single-row matmul stationaries against the ones row of the moving tile
    d_b2t = (nc.dram_tensor("b2t", [1, 2 * H], bf16, kind="ExternalInput")
             if b2nz else None)
    d_b3r = (nc.dram_tensor("b3r", [1, 2 * D + 2 * 2 * H], bf16, kind="ExternalInput")
             if b3nz else None)
    d_out = nc.dram_tensor("out", [D, Bs], f32, kind="ExternalOutput")

    DBG = bool(int(os.environ.get("KERNEL_DBG", "0")))
    d_dbg = {}
    if DBG:
        for nm, shp in (
            ("dbg_l1b_e0", [128, MJ * Ng]), ("dbg_sg1_e0", [128, MJ * Ng]),
            ("dbg_h1_e0", [128, MJ * Ng]), ("dbg_l2b_e0", [128, MJ * Ng]),
            ("dbg_h2_e0", [128, MJ * Ng]), ("dbg_acc_e0", [D, Ng]),
            ("dbg_l1b_e1", [128, MJ * Ng]), ("dbg_l1a_e1", [128, MJ * Ng]),
        ):
            d_dbg[nm] = nc.dram_tensor(nm, shp, f32, kind="ExternalOutput")

    PSB = int(os.environ.get("KERNEL_PSB", "7"))
    SGB = int(os.environ.get("KERNEL_SGB", "6"))
    HB = int(os.environ.get("KERNEL_HB", "6"))

    with tile.TileContext(nc) as tc:
        with (
            tc.tile_pool(name="const", bufs=1) as cpool,
            tc.tile_pool(name="psb", bufs=PSB, space="PSUM") as pspool,
            tc.tile_pool(name="pss", bufs=1, space="PSUM") as psspool,
            tc.tile_pool(name="sg", bufs=SGB) as sgpool,
            tc.tile_pool(name="h", bufs=HB) as hpool,
        ):
            if CTX8:
                tctxw = cpool.tile([C // 2, 2, G * Ng + 2 * H], f8)
                tctx = [tctxw[:, :, g * Ng:(g + 1) * Ng] for g in range(G)]
            else:
                tctxw = cpool.tile([C, G * Ng + 2 * H], bf16)
                tctx = [tctxw[:, g * Ng:(g + 1) * Ng] for g in range(G)]
            tthw = cpool.tile([DT2, G * Ng + 2 * H], bf16)
            tth = [tthw[:, g * Ng:(g + 1) * Ng] for g in range(G)]
            tthF = cpool.tile([D, G, Ng], f32)
            tw2 = cpool.tile([128, KCP, 2, 2 * H], f8)
            tw3v = cpool.tile([128, KCP, 2, 2, D], f8)
            tF = cpool.tile([128, KCP, 2, 2, 2 * H], f8)
            if b2nz:
                tb2 = cpool.tile([1, 2 * H], bf16)
            if b3nz:
                # cols: [b3 | 2*b3 | (dt/2)*b3@W1th | dt*b3@W1th]
                tb3 = cpool.tile([1, 2 * D + 2 * 2 * H], bf16)
            # both groups' accumulators share one bank; since start=True is
            # bank-granular, zeroing is done per step by an off-chain DVE
            # memset and every acc matmul uses start=False
            psaccb = psspool.tile([D, G, Ng], f32, name="accb")
            psacc = [psaccb[:, g, :] for g in range(G)]

            def w1c_col(half, j):
                base = G * Ng + half * H + j * 128
                if CTX8:
                    return tctxw[:, :, base:base + 128]
                return tctxw[:, base:base + 128]

            def w1tb_col(half, j):
                base = G * Ng + half * H + j * 128
                return tthw[:, base:base + 128]

            # startup DMAs: L1-critical tensors first, weights stream behind
            nc.sync.dma_start(tctxw[:], d_ctxw[:])
            nc.sync.dma_start(tthw[:], d_thw[:])
            nc.sync.dma_start(tthF[:], d_thF[:])
            for P in range(KCP):
                nc.sync.dma_start(tw2[:, P, :, :],
                                  d_w2[:, P * 2 * 2 * H:(P + 1) * 2 * 2 * H])
            nc.sync.dma_start(tw3v[:], d_w3v[:])
            nc.sync.dma_start(tF[:], d_fv[:])
            if b2nz:
                nc.sync.dma_start(tb2[:], d_b2t[:])
            if b3nz:
                nc.sync.dma_start(tb3[:], d_b3r[:])

            CUR = ["?"]

            def lab(inst, name):
                try:
                    OP_LABELS[inst.name] = name
                except Exception:
                    pass
                return inst

            def mm(out_ap, lhsT, rhs, start, stop, pm=None):
                lab(nc.tensor.matmul(out_ap, lhsT, rhs, start=start, stop=stop,
                                     perf_mode=pm), CUR[0])

            L1 = {}
            H1out = {}

            def issue_l1(g, close):
                """Pre-issue next eval's static L1 parts: ctx + (theta_s,
                t, b1) matmuls.  close=True ends the accumulation groups
                (step boundary, no F-term); else F matmuls close later."""
                bb = pspool.tile([128, MJ, Ng], f32, tag="bank", name=f"L1b{g}")
                ba = pspool.tile([128, MJ, Ng], f32, tag="bank", name=f"L1a{g}")
                CUR[0] = f"ctxMM.{g}"
                for half, bank in ((1, bb), (0, ba)):
                    for j in range(MJ):
                        # start only on the bank's first MM: a second start
                        # re-marks the whole bank pending-zero
                        mm(bank[:, j, :], w1c_col(half, j), tctx[g][:],
                           start=(j == 0), stop=False,
                           pm=(DR if CTX8 else None))
                CUR[0] = f"thMM.{g}"
                for half, bank in ((1, bb), (0, ba)):
                    for j in range(MJ):
                        mm(bank[:, j, :], w1tb_col(half, j), tth[g][:],
                           start=False, stop=close)
                return bb, ba

            def dbg_dump(nm, ap):
                if DBG and nm in d_dbg:
                    t = cpool.tile([ap.shape[0], int(np.prod(ap.shape[1:]))], f32,
                                   name=nm)
                    nc.scalar.copy(t[:], ap)
                    nc.sync.dma_start(d_dbg[nm][:], t[:])

            def phase1(g, e):
                # sigma1 over the whole b-bank, GLU1 -> fp8 pair tile
                bb, ba = L1[g]
                sg = sgpool.tile([128, MJ, Ng], bf16, tag="sg", name=f"sg1{g}")
                h1 = hpool.tile([128, MJ, Ng], f8, tag="h1", name=f"h1{g}")
                if SPLIT:
                    # per-pair halves: pair P's GLU output unblocks the P-MMs
                    # of L2 half an op earlier
                    for P in range(KCP):
                        sl = slice(2 * P, 2 * P + 2)
                        lab(nc.scalar.activation(sg[:, sl, :], bb[:, sl, :],
                                                 SIGMOID), f"sig1{P}.{g}.{e}")
                        lab(nc.vector.scalar_tensor_tensor(
                            h1[:, sl, :], ba[:, sl, :], 1.0, sg[:, sl, :],
                            ALU.mult, ALU.mult), f"glu1{P}.{g}.{e}")
                else:
                    lab(nc.scalar.activation(sg[:], bb[:, :, :], SIGMOID), f"sig1.{g}.{e}")
                    lab(nc.vector.scalar_tensor_tensor(h1[:], ba[:, :, :], 1.0, sg[:],
                                                       ALU.mult, ALU.mult), f"glu1.{g}.{e}")
                H1out[g] = h1
                if DBG and g == 0 and e == 0:
                    dbg_dump("dbg_l1b_e0", bb[:, :, :])
                    dbg_dump("dbg_sg1_e0", sg[:])
                    dbg_dump("dbg_h1_e0", h1[:])
                if DBG and g == 0 and e == 1:
                    dbg_dump("dbg_l1b_e1", bb[:, :, :])
                    dbg_dump("dbg_l1a_e1", ba[:, :, :])

            L2banks = {}

            def phase2L(g, e):
                # L2 matmuls + sigma2 (L2a runs behind sigma2 on the PE)
                h1 = H1out[g]
                CUR[0] = f"L2MM.{g}.{e}"
                bb2 = pspool.tile([128, MJ, Ng], f32, tag="bank", name=f"L2b{g}")
                ba2 = pspool.tile([128, MJ, Ng], f32, tag="bank", name=f"L2a{g}")
                for P in range(KCP):     # P-major: P0 matmuls only need h1-P0
                    for j in range(MJ):
                        mm(bb2[:, j, :], tw2[:, P, :, H + j * 128:H + (j + 1) * 128],
                           h1[:, 2 * P:2 * P + 2, :],
                           start=(j == 0 and P == 0),
                           stop=(P == KCP - 1 and not b2nz), pm=DR)
                if b2nz:  # fallback: bias via ones-row matmuls (b-half)
                    for j in range(MJ):
                        mm(bb2[:, j, :], tb2[:, H + j * 128:H + (j + 1) * 128],
                           tth[g][D + 1:D + 2, :], start=False, stop=True)
                sg2 = sgpool.tile([128, MJ, Ng], bf16, tag="sg", name=f"sg2{g}")
                if SPLIT and not b2nz:
                    # chunk-split sigma2: the first half unblocks GLU2's first
                    # half; b-bank layout is j-major so split by j pairs
                    for hj in range(2):
                        sl = slice(2 * hj, 2 * hj + 2)
                        lab(nc.scalar.activation(sg2[:, sl, :], bb2[:, sl, :],
                                                 SIGMOID), f"sig2{hj}.{g}.{e}")
                else:
                    lab(nc.scalar.activation(sg2[:], bb2[:, :, :], SIGMOID), f"sig2.{g}.{e}")
                for P in range(KCP):
                    for j in range(MJ):
                        mm(ba2[:, j, :], tw2[:, P, :, j * 128:(j + 1) * 128],
                           h1[:, 2 * P:2 * P + 2, :],
                           start=(j == 0 and P == 0),
                           stop=(P == KCP - 1 and not b2nz), pm=DR)
                if b2nz:
                    for j in range(MJ):
                        mm(ba2[:, j, :], tb2[:, j * 128:(j + 1) * 128],
                           tth[g][D + 1:D + 2, :], start=False, stop=True)
                L2banks[g] = (bb2, ba2, sg2)

            def pre_issue(g, e):
                # static L1 parts of eval e (ctx + theta_s + t + b1) — no data
                # deps beyond tth/tctx, so these fill PE dependency-wait gaps.
                # Skipped for e%4==0 (issued in phase2T after the state copy).
                nc.gpsimd.memset(tth[g][D:D + 1, :], float(TVAL[e]))
                L1[g] = issue_l1(g, close=False)

            def phase2T(g, e):
                # GLU2 -> acc matmuls -> F-shortcut (or boundary state update)
                s, ei = divmod(e, 4)
                last = e == NEV - 1
                boundary = ei == 3
                bb2, ba2, sg2 = L2banks[g]
                h2 = hpool.tile([128, MJ, Ng], f8, tag="h2", name=f"h2{g}")
                if SPLIT:
                    for hj in range(2):
                        sl = slice(2 * hj, 2 * hj + 2)
                        lab(nc.vector.scalar_tensor_tensor(
                            h2[:, sl, :], ba2[:, sl, :], S_H, sg2[:, sl, :],
                            ALU.mult, ALU.mult), f"glu2{hj}.{g}.{e}")
                else:
                    lab(nc.vector.scalar_tensor_tensor(h2[:], ba2[:, :, :], S_H, sg2[:],
                                                       ALU.mult, ALU.mult), f"glu2.{g}.{e}")
                acc = psacc[g]
                v = 0 if ei in (0, 3) else 1
                if ei == 0:
                    lab(nc.vector.memset(acc, 0.0), f"accz.{g}.{e}")

                def acc_mms():
                    # RK4 accumulator: acc += w_e * k_e (pre-scaled W3)
                    CUR[0] = f"accMM.{g}.{e}"
                    for P in range(KCP):
                        mm(acc, tw3v[:, P, :, v, :], h2[:, 2 * P:2 * P + 2, :],
                           start=False,
                           stop=(P == KCP - 1 and not b3nz), pm=DR)
                    if b3nz:
                        boff = 0 if ei in (0, 3) else D
                        mm(acc, tb3[:, boff:boff + D],
                           tth[g][D + 1:D + 2, :], start=False, stop=True)

                if boundary:
                    acc_mms()   # thSTT depends on acc: keep it first
                if DBG and g == 0 and e == 0:
                    dbg_dump("dbg_l2b_e0", bb2[:, :, :])
                    dbg_dump("dbg_h2_e0", h2[:])
                if not boundary:
                    # F-shortcut: theta-correction of the NEXT eval's L1 pre-
                    # activations directly from h2 (closes the L1 banks);
                    # b-bank first so sigma1 unblocks before GLU1 needs a-bank
                    CUR[0] = f"FMM.{g}.{e}"
                    fv = 0 if ei < 2 else 1
                    bb, ba = L1[g]
                    for P in range(KCP):   # P-major: P0 runs off GLU2's 1st half
                        for half, bank in ((1, bb), (0, ba)):
                            for j in range(MJ):
                                col = half * H + j * 128
                                mm(bank[:, j, :],
                                   tF[:, P, :, fv, col:col + 128],
                                   h2[:, 2 * P:2 * P + 2, :],
                                   start=False,
                                   stop=(P == KCP - 1 and not b3nz), pm=DR)
                    if b3nz:
                        # c_e * (b3 @ W1theta) correction row via the ones row
                        for half, bank in ((1, bb), (0, ba)):
                            for j in range(MJ):
                                boff = 2 * D + fv * 2 * H + half * H + j * 128
                                mm(bank[:, j, :], tb3[:, boff:boff + 128],
                                   tth[g][D + 1:D + 2, :], start=False, stop=True)
                    acc_mms()  # off the critical chain on fast boundaries
                elif last:
                    nc.vector.scalar_tensor_tensor(
                        tthF[:, g, :], acc, float(dt / 6.0), tthF[:, g, :],
                        ALU.mult, ALU.add)
                else:
                    # step boundary.  Critical chain: acc -> bf16 theta tile
                    # -> theta matmuls -> sigma1(e+1).  The f32 state update
                    # (same inputs) runs behind it, off-chain.
                    lab(nc.vector.scalar_tensor_tensor(
                        tth[g][0:D, :], acc, float(dt / 6.0), tthF[:, g, :],
                        ALU.mult, ALU.add), f"thSTT.{g}.{e}")
                    nc.gpsimd.memset(tth[g][D:D + 1, :], float(TVAL[e + 1]))
                    L1[g] = issue_l1(g, close=True)
                    nc.vector.scalar_tensor_tensor(
                        tthF[:, g, :], acc, float(dt / 6.0), tthF[:, g, :],
                        ALU.mult, ALU.add)

            # ---- prologue: first-eval static parts for both groups ----
            L1[0] = issue_l1(0, close=True)
            L1[1] = issue_l1(1, close=True)

            # ---- slot walk: B lags A by a quarter period.  Per slot the
            # engine streams are ACT [s1A, s2B, s2A, s1B], DVE [g1A, g2B,
            # g2A, g1B], PE [L2B, preB', L2A, accB+FB, preA', accA+FA] so
            # neither group's chain waits on the other's long segments. ----
            for e in range(NEV):
                phase1(0, e)
                if e > 0:
                    phase2L(1, e - 1)
                    if (e - 1) % 4 != 3:
                        pre_issue(1, e)
                phase2L(0, e)
                if e > 0:
                    phase2T(1, e - 1)
                if e % 4 != 3:
                    pre_issue(0, e + 1)
                phase2T(0, e)
                phase1(1, e)
            phase2L(1, NEV - 1)
            phase2T(1, NEV - 1)

            nc.sync.dma_start(d_out[:], tthF[:, :, :])

    # ---- per-core input maps ----
    w1c_b = w1c_h.astype(_bf16)
    w1tb_b = w1tb_h.astype(_bf16)
    w2_q = w2_h.astype(_f8np)
    w3v_q = w3v_h.astype(_f8np)
    fv_q = fv_h.astype(_f8np)
    if b2nz:
        b2t_h = np.ascontiguousarray(b2f.reshape(1, 2 * H)).astype(_bf16)
    if b3nz:
        b3w = (b3f @ W1f[0:D]).reshape(1, 2 * H)
        b3r_h = np.concatenate(
            [b3f.reshape(1, D), 2.0 * b3f.reshape(1, D),
             0.5 * dt * b3w, dt * b3w], axis=1).astype(_bf16)
    in_maps = []
    for c in range(N_CORES):
        sl = slice(c * Bs, (c + 1) * Bs)
        th_T = np.ascontiguousarray(np.asarray(theta0[sl], np.float32).T)  # [32,256]
        ctx_T = np.ascontiguousarray(np.asarray(context[sl], np.float32).T)  # [128,256]
        thg = []
        for g in range(G):
            t34 = np.zeros((DT2, Ng), np.float32)
            t34[0:D] = th_T[:, g * Ng:(g + 1) * Ng]
            t34[D] = 0.0          # t row (t=0 at start)
            t34[D + 1] = 1.0      # ones row
            thg.append(t34)
        thw = np.ascontiguousarray(np.concatenate(
            [np.concatenate(thg, axis=1).astype(_bf16), w1tb_b], axis=1))
        if CTX8:
            ctx_dr = ctx_T.reshape(2, C // 2, Bs).transpose(1, 0, 2)
            ctxw = np.ascontiguousarray(np.concatenate(
                [ctx_dr, w1c8_h], axis=2).reshape(C // 2, -1)).astype(_f8np)
        else:
            ctxw = np.ascontiguousarray(np.concatenate(
                [ctx_T.astype(_bf16), w1c_b], axis=1))
        m = {
            "ctxw": ctxw,
            "thw": thw,
            "thF": th_T,
            "w2": w2_q,
            "w3v": w3v_q,
            "fv": fv_q,
        }
        if b2nz:
            m["b2t"] = b2t_h
        if b3nz:
            m["b3r"] = b3r_h
        in_maps.append(m)

    return nc, in_maps


def _build_and_run(theta0, context, W1, b1, W2, b2, W3, b3, n_steps):
    from concourse.bass_utils import run_bass_kernel_spmd

    nc, in_maps = _build_program(theta0, context, W1, b1, W2, b2, W3, b3, n_steps)
    nc.finalize()
    res = run_bass_kernel_spmd(
        nc,
        in_maps,
        core_ids=list(range(N_CORES)),
        trace=bool(int(os.environ.get("KERNEL_TRACE", "0"))),
    )
    _build_and_run.last_results = res

    out = np.concatenate([r["out"].T for r in res.results], axis=0)
    return np.ascontiguousarray(out.astype(np.float32))


def kernel(theta0, context, W1, b1, W2, b2, W3, b3, n_steps):
    return _build_and_run(
        np.asarray(theta0), np.asarray(context), W1, b1, W2, b2, W3, b3, n_steps
    )


# revision 31
# speedup vs baseline: 1.0092x; 1.0092x over previous
"""CCNF RK4 sampling kernel for 8 Trainium2 NeuronCores — v2.

Data-parallel across cores (2048 -> 256/core), and each core's batch is
split into TWO groups of 128 samples whose serial RK4 chains are
software-pipelined half-an-eval apart, so one group's L1 sigmoid/GLU
phase overlaps the other group's L2/L3 phase on the ACT/DVE engines.

The v1 kernel was latency-bound on the per-eval serial chain
(theta-MM -> 4x(sigma,GLU) -> L2 -> 4x(sigma,GLU) -> L3 -> RK4-STT ->
theta-MM', ~6.9us/eval).  v2 shortens the chain per group and hides the
rest with the second group:

  - whole-bank ops: sigma is ONE activation op per layer over a full
    [128, 4, 128] PSUM bank (4 chunks), GLU is ONE STT.  Bias made
    unnecessary: the time row t*W1[32] + b1 ride the theta-stationary
    ([34, 128]: theta rows + t row + ones row, maintained by gpsimd
    memsets on the idle Pool engine).
  - F-shortcut: tx = theta_s + c*k feeds L1 only through W1theta, so
    L1pre(e+1) = [ctx + theta_s + t] (pre-issued off-chain) +
    h2_e @ Fc where F = W3 @ W1[0:32] is precomputed host-side and
    applied as fp8 DoubleRow matmuls.  This removes L3->STT->theta-MM
    (two sem hops + a DVE op) from 3 of 4 eval boundaries.
  - RK4 combination in PSUM: acc += w_e * k_e via duplicate cheap L3
    DR matmuls with pre-scaled W3 variants; one STT per STEP updates
    the f32 theta state (thF), one ACT copy refreshes the bf16
    matmul-input copy.  (v1 spent 2 DVE STTs per eval here.)
  - fp8 scales: h2 is written scaled by s_h=1/4 (free in the GLU STT
    scalar) so Fc = F*c/s_h and w3 variants stay in fp8 normal range.

Numpy-probed accuracy of this exact quantization pipeline: 1.13e-2
(gate 2e-2).  Cost model: ~3.8us per eval-pair vs v1's 6.9us per eval.
"""

import os

import numpy as np
from ml_dtypes import bfloat16 as _bf16
from ml_dtypes import float8_e4m3 as _f8np

N_CORES = 8
G = 2          # pipelined sample groups per core
CTX8 = bool(int(os.environ.get("KERNEL_CTX8", "0")))  # fp8 DoubleRow ctx matmuls
SPLIT = bool(int(os.environ.get("KERNEL_SPLIT", "1")))  # pair-split sigma/GLU ops
OP_LABELS = {}  # instruction name -> human label (for the trace analyzer)
S_H = 0.25     # h2 scale carried in the GLU2 STT scalar


def _build_program(theta0, context, W1, b1, W2, b2, W3, b3, n_steps):
    import concourse.mybir as mybir
    import concourse.tile as tile
    from concourse import bacc

    f32 = mybir.dt.float32
    f32r = mybir.dt.float32r
    bf16 = mybir.dt.bfloat16
    f8 = mybir.dt.float8e4
    DR = mybir.MatmulPerfMode.DoubleRow
    ALU = mybir.AluOpType
    SIGMOID = mybir.ActivationFunctionType.Sigmoid

    B, D = theta0.shape          # 2048, 32
    C = context.shape[1]         # 128
    IN, H2 = W1.shape            # 161, 1024
    H = W2.shape[0]              # 512
    assert H2 == 2 * H and W2.shape[1] == 2 * H and W3.shape == (H, D)
    assert IN == D + 1 + C
    assert B % (N_CORES * G) == 0
    Bs = B // N_CORES            # 256
    Ng = Bs // G                 # 128
    steps = int(n_steps)
    dt = 1.0 / steps
    MJ = H // 128                # 4 column chunks per GLU half
    KCP = MJ // 2                # 2 DoubleRow pairs over the H contraction
    NEV = 4 * steps
    DT2 = D + 2                  # moving rows: theta(32) + t(1) + ones(1)

    b2f = np.asarray(b2, np.float32)
    b3f = np.asarray(b3, np.float32)
    b2nz = bool(np.any(b2f))
    b3nz = bool(np.any(b3f))

    # t value per eval (t = idx * dt/2)
    IOFF = (0, 1, 1, 2)
    TVAL = [(2 * (e // 4) + IOFF[e % 4]) * (dt / 2.0) for e in range(NEV + 1)]

    # ---- host-side layout prep (shared across cores) ----
    W1f = np.asarray(W1, np.float32)
    w1c_h = np.ascontiguousarray(W1f[D + 1:])                    # [128, 1024]
    if CTX8:
        # DoubleRow pairing along the 128 ctx rows: k = plane*64 + p
        w1c8_h = np.ascontiguousarray(
            w1c_h.reshape(2, C // 2, 2 * H).transpose(1, 0, 2))  # [64, 2, 1024]
    w1tb_h = np.concatenate(
        [W1f[0:D + 1], np.asarray(b1, np.float32).reshape(1, 2 * H)], axis=0
    )                                                            # [34, 1024]
    w2_h = np.ascontiguousarray(
        np.asarray(W2, np.float32)
        .reshape(KCP, 2, 128, 2 * H).transpose(2, 0, 1, 3)
        .reshape(128, KCP * 2 * 2 * H)
    )

    W3f = np.asarray(W3, np.float32)

    def drpack(w, ncol):  # [H, ncol] -> [128, KCP, 2, ncol]
        return w.reshape(KCP, 2, 128, ncol).transpose(2, 0, 1, 3)

    # w3 variants scaled by w_e / s_h (w_e in {1, 2})
    w3v_h = np.ascontiguousarray(np.stack(
        [drpack(W3f * (1.0 / S_H), D), drpack(W3f * (2.0 / S_H), D)], axis=3
    ).reshape(128, KCP * 2 * 2 * D))                             # [128, P, pl, v, D]
    # F variants scaled by c_e / s_h (c_e in {dt/2, dt})
    F_h = W3f @ W1f[0:D]                                         # [512, 1024]
    fv_h = np.ascontiguousarray(np.stack(
        [drpack(F_h * (0.5 * dt / S_H), 2 * H), drpack(F_h * (dt / S_H), 2 * H)],
        axis=3,
    ).reshape(128, KCP * 2 * 2 * 2 * H))                         # [128, P, pl, v, 1024]

    # ---- build the bass program (same program on all 8 cores) ----
    nc = bacc.Bacc("TRN2", target_bir_lowering=False)

    if CTX8:
        d_ctxw = nc.dram_tensor("ctxw", [C // 2, 2 * (G * Ng + 2 * H)], f8,
                                kind="ExternalInput")
    else:
        d_ctxw = nc.dram_tensor("ctxw", [C, G * Ng + 2 * H], bf16,
                                kind="ExternalInput")
    d_thw = nc.dram_tensor("thw", [DT2, G * Ng + 2 * H], bf16, kind="ExternalInput")
    d_thF = nc.dram_tensor("thF", [D, Bs], f32, kind="ExternalInput")
    d_w2 = nc.dram_tensor("w2", [128, KCP * 2 * 2 * H], f8, kind="ExternalInput")
    d_w3v = nc.dram_tensor("w3v", [128, KCP * 2 * 2 * D], f8, kind="ExternalInput")
    d_fv = nc.dram_tensor("fv", [128, KCP * 2 * 2 * 2 * H], f8, kind="ExternalInput")
    # bias fallbacks (all-zero in the reference problem): bias values ride as
    # single-row matmul stationaries against the ones row of the moving tile
    d_b2t = (nc.dram_tensor("b2t", [1, 2 * H], bf16, kind="ExternalInput")
             if b2nz else None)
    d_b3r = (nc.dram_tensor("b3r", [1, 2 * D + 2 * 2 * H], bf16, kind="ExternalInput")
             if b3nz else None)
    d_out = nc.dram_tensor("out", [D, Bs], f32, kind="ExternalOutput")

    DBG = bool(int(os.environ.get("KERNEL_DBG", "0")))
    d_dbg = {}
    if DBG:
        for nm, shp in (
            ("dbg_l1b_e0", [128, MJ * Ng]), ("dbg_sg1_e0", [128, MJ * Ng]),
            ("dbg_h1_e0", [128, MJ * Ng]), ("dbg_l2b_e0", [128, MJ * Ng]),
            ("dbg_h2_e0", [128, MJ * Ng]), ("dbg_acc_e0", [D, Ng]),
            ("dbg_l1b_e1", [128, MJ * Ng]), ("dbg_l1a_e1", [128, MJ * Ng]),
        ):
            d_dbg[nm] = nc.dram_tensor(nm, shp, f32, kind="ExternalOutput")

    PSB = int(os.environ.get("KERNEL_PSB", "6"))
    SGB = int(os.environ.get("KERNEL_SGB", "6"))
    HB = int(os.environ.get("KERNEL_HB", "6"))

    with tile.TileContext(nc) as tc:
        with (
            tc.tile_pool(name="const", bufs=1) as cpool,
            tc.tile_pool(name="psb", bufs=PSB, space="PSUM") as pspool,
            tc.tile_pool(name="pss", bufs=1, space="PSUM") as psspool,
            tc.tile_pool(name="sg", bufs=SGB) as sgpool,
            tc.tile_pool(name="h", bufs=HB) as hpool,
        ):
            if CTX8:
                tctxw = cpool.tile([C // 2, 2, G * Ng + 2 * H], f8)
                tctx = [tctxw[:, :, g * Ng:(g + 1) * Ng] for g in range(G)]
            else:
                tctxw = cpool.tile([C, G * Ng + 2 * H], bf16)
                tctx = [tctxw[:, g * Ng:(g + 1) * Ng] for g in range(G)]
            tthw = cpool.tile([DT2, G * Ng + 2 * H], bf16)
            tth = [tthw[:, g * Ng:(g + 1) * Ng] for g in range(G)]
            tthF = cpool.tile([D, G, Ng], f32)
            tw2 = cpool.tile([128, KCP, 2, 2 * H], f8)
            tw3v = cpool.tile([128, KCP, 2, 2, D], f8)
            tF = cpool.tile([128, KCP, 2, 2, 2 * H], f8)
            if b2nz:
                tb2 = cpool.tile([1, 2 * H], bf16)
            if b3nz:
                # cols: [b3 | 2*b3 | (dt/2)*b3@W1th | dt*b3@W1th]
                tb3 = cpool.tile([1, 2 * D + 2 * 2 * H], bf16)
            # one acc bank per group: PSUM start=True is bank-granular, so
            # the accumulators cannot share a bank with anything live
            psacc = [psspool.tile([D, Ng], f32, name=f"acc{g}") for g in range(G)]

            def w1c_col(half, j):
                base = G * Ng + half * H + j * 128
                if CTX8:
                    return tctxw[:, :, base:base + 128]
                return tctxw[:, base:base + 128]

            def w1tb_col(half, j):
                base = G * Ng + half * H + j * 128
                return tthw[:, base:base + 128]

            # startup DMAs: L1-critical tensors first, weights stream behind
            nc.sync.dma_start(tctxw[:], d_ctxw[:])
            nc.sync.dma_start(tthw[:], d_thw[:])
            nc.sync.dma_start(tthF[:], d_thF[:])
            for P in range(KCP):
                nc.sync.dma_start(tw2[:, P, :, :],
                                  d_w2[:, P * 2 * 2 * H:(P + 1) * 2 * 2 * H])
            nc.sync.dma_start(tw3v[:], d_w3v[:])
            nc.sync.dma_start(tF[:], d_fv[:])
            if b2nz:
                nc.sync.dma_start(tb2[:], d_b2t[:])
            if b3nz:
                nc.sync.dma_start(tb3[:], d_b3r[:])

            CUR = ["?"]

            def lab(inst, name):
                try:
                    OP_LABELS[inst.name] = name
                except Exception:
                    pass
                return inst

            def mm(out_ap, lhsT, rhs, start, stop, pm=None):
                lab(nc.tensor.matmul(out_ap, lhsT, rhs, start=start, stop=stop,
                                     perf_mode=pm), CUR[0])

            L1 = {}
            H1out = {}

            def issue_l1(g, close):
                """Pre-issue next eval's static L1 parts: ctx + (theta_s,
                t, b1) matmuls.  close=True ends the accumulation groups
                (step boundary, no F-term); else F matmuls close later."""
                bb = pspool.tile([128, MJ, Ng], f32, tag="bank", name=f"L1b{g}")
                ba = pspool.tile([128, MJ, Ng], f32, tag="bank", name=f"L1a{g}")
                CUR[0] = f"ctxMM.{g}"
                for half, bank in ((1, bb), (0, ba)):
                    for j in range(MJ):
                        # start only on the bank's first MM: a second start
                        # re-marks the whole bank pending-zero
                        mm(bank[:, j, :], w1c_col(half, j), tctx[g][:],
                           start=(j == 0), stop=False,
                           pm=(DR if CTX8 else None))
                CUR[0] = f"thMM.{g}"
                for half, bank in ((1, bb), (0, ba)):
                    for j in range(MJ):
                        mm(bank[:, j, :], w1tb_col(half, j), tth[g][:],
                           start=False, stop=close)
                return bb, ba

            def dbg_dump(nm, ap):
                if DBG and nm in d_dbg:
                    t = cpool.tile([ap.shape[0], int(np.prod(ap.shape[1:]))], f32,
                                   name=nm)
                    nc.scalar.copy(t[:], ap)
                    nc.sync.dma_start(d_dbg[nm][:], t[:])

            def phase1(g, e):
                # sigma1 over the whole b-bank, GLU1 -> fp8 pair tile
                bb, ba = L1[g]
                sg = sgpool.tile([128, MJ, Ng], bf16, tag="sg", name=f"sg1{g}")
                h1 = hpool.tile([128, MJ, Ng], f8, tag="h1", name=f"h1{g}")
                if SPLIT:
                    # per-pair halves: pair P's GLU output unblocks the P-MMs
                    # of L2 half an op earlier
                    for P in range(KCP):
                        sl = slice(2 * P, 2 * P + 2)
                        lab(nc.scalar.activation(sg[:, sl, :], bb[:, sl, :],
                                                 SIGMOID), f"sig1{P}.{g}.{e}")
                        lab(nc.vector.scalar_tensor_tensor(
                            h1[:, sl, :], ba[:, sl, :], 1.0, sg[:, sl, :],
                            ALU.mult, ALU.mult), f"glu1{P}.{g}.{e}")
                else:
                    lab(nc.scalar.activation(sg[:], bb[:, :, :], SIGMOID), f"sig1.{g}.{e}")
                    lab(nc.vector.scalar_tensor_tensor(h1[:], ba[:, :, :], 1.0, sg[:],
                                                       ALU.mult, ALU.mult), f"glu1.{g}.{e}")
                H1out[g] = h1
                if DBG and g == 0 and e == 0:
                    dbg_dump("dbg_l1b_e0", bb[:, :, :])
                    dbg_dump("dbg_sg1_e0", sg[:])
                    dbg_dump("dbg_h1_e0", h1[:])
                if DBG and g == 0 and e == 1:
                    dbg_dump("dbg_l1b_e1", bb[:, :, :])
                    dbg_dump("dbg_l1a_e1", ba[:, :, :])

            L2banks = {}

            def phase2L(g, e):
                # L2 matmuls + sigma2 (L2a runs behind sigma2 on the PE)
                h1 = H1out[g]
                CUR[0] = f"L2MM.{g}.{e}"
                bb2 = pspool.tile([128, MJ, Ng], f32, tag="bank", name=f"L2b{g}")
                ba2 = pspool.tile([128, MJ, Ng], f32, tag="bank", name=f"L2a{g}")
                for P in range(KCP):     # P-major: P0 matmuls only need h1-P0
                    for j in range(MJ):
                        mm(bb2[:, j, :], tw2[:, P, :, H + j * 128:H + (j + 1) * 128],
                           h1[:, 2 * P:2 * P + 2, :],
                           start=(j == 0 and P == 0),
                           stop=(P == KCP - 1 and not b2nz), pm=DR)
                if b2nz:  # fallback: bias via ones-row matmuls (b-half)
                    for j in range(MJ):
                        mm(bb2[:, j, :], tb2[:, H + j * 128:H + (j + 1) * 128],
                           tth[g][D + 1:D + 2, :], start=False, stop=True)
                sg2 = sgpool.tile([128, MJ, Ng], bf16, tag="sg", name=f"sg2{g}")
                if SPLIT and not b2nz:
                    # chunk-split sigma2: the first half unblocks GLU2's first
                    # half; b-bank layout is j-major so split by j pairs
                    for hj in range(2):
                        sl = slice(2 * hj, 2 * hj + 2)
                        lab(nc.scalar.activation(sg2[:, sl, :], bb2[:, sl, :],
                                                 SIGMOID), f"sig2{hj}.{g}.{e}")
                else:
                    lab(nc.scalar.activation(sg2[:], bb2[:, :, :], SIGMOID), f"sig2.{g}.{e}")
                for P in range(KCP):
                    for j in range(MJ):
                        mm(ba2[:, j, :], tw2[:, P, :, j * 128:(j + 1) * 128],
                           h1[:, 2 * P:2 * P + 2, :],
                           start=(j == 0 and P == 0),
                           stop=(P == KCP - 1 and not b2nz), pm=DR)
                if b2nz:
                    for j in range(MJ):
                        mm(ba2[:, j, :], tb2[:, j * 128:(j + 1) * 128],
                           tth[g][D + 1:D + 2, :], start=False, stop=True)
                L2banks[g] = (bb2, ba2, sg2)

            def pre_issue(g, e):
                # static L1 parts of eval e (ctx + theta_s + t + b1) — no data
                # deps beyond tth/tctx, so these fill PE dependency-wait gaps.
                # Skipped for e%4==0 (issued in phase2T after the state copy).
                nc.gpsimd.memset(tth[g][D:D + 1, :], float(TVAL[e]))
                L1[g] = issue_l1(g, close=False)

            def phase2T(g, e):
                # GLU2 -> acc matmuls -> F-shortcut (or boundary state update)
                s, ei = divmod(e, 4)
                last = e == NEV - 1
                boundary = ei == 3
                bb2, ba2, sg2 = L2banks[g]
                h2 = hpool.tile([128, MJ, Ng], f8, tag="h2", name=f"h2{g}")
                if SPLIT:
                    for hj in range(2):
                        sl = slice(2 * hj, 2 * hj + 2)
                        lab(nc.vector.scalar_tensor_tensor(
                            h2[:, sl, :], ba2[:, sl, :], S_H, sg2[:, sl, :],
                            ALU.mult, ALU.mult), f"glu2{hj}.{g}.{e}")
                else:
                    lab(nc.vector.scalar_tensor_tensor(h2[:], ba2[:, :, :], S_H, sg2[:],
                                                       ALU.mult, ALU.mult), f"glu2.{g}.{e}")
                acc = psacc[g][:]
                v = 0 if ei in (0, 3) else 1

                def acc_mms():
                    # RK4 accumulator: acc += w_e * k_e (pre-scaled W3)
                    CUR[0] = f"accMM.{g}.{e}"
                    for P in range(KCP):
                        mm(acc, tw3v[:, P, :, v, :], h2[:, 2 * P:2 * P + 2, :],
                           start=(ei == 0 and P == 0),
                           stop=(P == KCP - 1 and not b3nz), pm=DR)
                    if b3nz:
                        boff = 0 if ei in (0, 3) else D
                        mm(acc, tb3[:, boff:boff + D],
                           tth[g][D + 1:D + 2, :], start=False, stop=True)

                if boundary:
                    acc_mms()   # thSTT depends on acc: keep it first
                if DBG and g == 0 and e == 0:
                    dbg_dump("dbg_l2b_e0", bb2[:, :, :])
                    dbg_dump("dbg_h2_e0", h2[:])
                if not boundary:
                    # F-shortcut: theta-correction of the NEXT eval's L1 pre-
                    # activations directly from h2 (closes the L1 banks);
                    # b-bank first so sigma1 unblocks before GLU1 needs a-bank
                    CUR[0] = f"FMM.{g}.{e}"
                    fv = 0 if ei < 2 else 1
                    bb, ba = L1[g]
                    for P in range(KCP):   # P-major: P0 runs off GLU2's 1st half
                        for half, bank in ((1, bb), (0, ba)):
                            for j in range(MJ):
                                col = half * H + j * 128
                                mm(bank[:, j, :],
                                   tF[:, P, :, fv, col:col + 128],
                                   h2[:, 2 * P:2 * P + 2, :],
                                   start=False,
                                   stop=(P == KCP - 1 and not b3nz), pm=DR)
                    if b3nz:
                        # c_e * (b3 @ W1theta) correction row via the ones row
                        for half, bank in ((1, bb), (0, ba)):
                            for j in range(MJ):
                                boff = 2 * D + fv * 2 * H + half * H + j * 128
                                mm(bank[:, j, :], tb3[:, boff:boff + 128],
                                   tth[g][D + 1:D + 2, :], start=False, stop=True)
                    acc_mms()  # off the critical chain on fast boundaries
                elif last:
                    nc.vector.scalar_tensor_tensor(
                        tthF[:, g, :], acc, float(dt / 6.0), tthF[:, g, :],
                        ALU.mult, ALU.add)
                else:
                    # step boundary.  Critical chain: acc -> bf16 theta tile
                    # -> theta matmuls -> sigma1(e+1).  The f32 state update
                    # (same inputs) runs behind it, off-chain.
                    lab(nc.vector.scalar_tensor_tensor(
                        tth[g][0:D, :], acc, float(dt / 6.0), tthF[:, g, :],
                        ALU.mult, ALU.add), f"thSTT.{g}.{e}")
                    nc.gpsimd.memset(tth[g][D:D + 1, :], float(TVAL[e + 1]))
                    L1[g] = issue_l1(g, close=True)
                    nc.vector.scalar_tensor_tensor(
                        tthF[:, g, :], acc, float(dt / 6.0), tthF[:, g, :],
                        ALU.mult, ALU.add)

            # ---- prologue: first-eval static parts for both groups ----
            L1[0] = issue_l1(0, close=True)
            L1[1] = issue_l1(1, close=True)

            # ---- slot walk: B lags A by a quarter period.  Per slot the
            # engine streams are ACT [s1A, s2B, s2A, s1B], DVE [g1A, g2B,
            # g2A, g1B], PE [L2B, preB', L2A, accB+FB, preA', accA+FA] so
            # neither group's chain waits on the other's long segments. ----
            for e in range(NEV):
                phase1(0, e)
                if e > 0:
                    phase2L(1, e - 1)
                    if (e - 1) % 4 != 3:
                        pre_issue(1, e)
                phase2L(0, e)
                if e > 0:
                    phase2T(1, e - 1)
                if e % 4 != 3:
                    pre_issue(0, e + 1)
                phase2T(0, e)
                phase1(1, e)
            phase2L(1, NEV - 1)
            phase2T(1, NEV - 1)

            nc.sync.dma_start(d_out[:], tthF[:, :, :])

    # ---- per-core input maps ----
    w1c_b = w1c_h.astype(_bf16)
    w1tb_b = w1tb_h.astype(_bf16)
    w2_q = w2_h.astype(_f8np)
    w3v_q = w3v_h.astype(_f8np)
    fv_q = fv_h.astype(_f8np)
    if b2nz:
        b2t_h = np.ascontiguousarray(b2f.reshape(1, 2 * H)).astype(_bf16)
    if b3nz:
        b3w = (b3f @ W1f[0:D]).reshape(1, 2 * H)
        b3r_h = np.concatenate(
            [b3f.reshape(1, D), 2.0 * b3f.reshape(1, D),
             0.5 * dt * b3w, dt * b3w], axis=1).astype(_bf16)
    in_maps = []
    for c in range(N_CORES):
        sl = slice(c * Bs, (c + 1) * Bs)
        th_T = np.ascontiguousarray(np.asarray(theta0[sl], np.float32).T)  # [32,256]
        ctx_T = np.ascontiguousarray(np.asarray(context[sl], np.float32).T)  # [128,256]
        thg = []
        for g in range(G):
            t34 = np.zeros((DT2, Ng), np.float32)
            t34[0:D] = th_T[:, g * Ng:(g + 1) * Ng]
            t34[D] = 0.0          # t row (t=0 at start)
            t34[D + 1] = 1.0      # ones row
            thg.append(t34)
        thw = np.ascontiguousarray(np.concatenate(
            [np.concatenate(thg, axis=1).astype(_bf16), w1tb_b], axis=1))
        if CTX8:
            ctx_dr = ctx_T.reshape(2, C // 2, Bs).transpose(1, 0, 2)
            ctxw = np.ascontiguousarray(np.concatenate(
                [ctx_dr, w1c8_h], axis=2).reshape(C // 2, -1)).astype(_f8np)
        else:
            ctxw = np.ascontiguousarray(np.concatenate(
                [ctx_T.astype(_bf16), w1c_b], axis=1))
        m = {
            "ctxw": ctxw,
            "thw": thw,
            "thF": th_T,
            "w2": w2_q,
            "w3v": w3v_q,
            "fv": fv_q,
        }
        if b2nz:
            m["b2t"] = b2t_h
        if b3nz:
            m["b3r"] = b3r_h
        in_maps.append(m)

    return nc, in_maps


def _build_and_run(theta0, context, W1, b1, W2, b2, W3, b3, n_steps):
    from concourse.bass_utils import run_bass_kernel_spmd

    nc, in_maps = _build_program(theta0, context, W1, b1, W2, b2, W3, b3, n_steps)
    nc.finalize()
    res = run_bass_kernel_spmd(
        nc,
        in_maps,
        core_ids=list(range(N_CORES)),
        trace=bool(int(os.environ.get("KERNEL_TRACE", "0"))),
    )
    _build_and_run.last_results = res

    out = np.concatenate([r["out"].T for r in res.results], axis=0)
    return np.ascontiguousarray(out.astype(np.float32))


def kernel(theta0, context, W1, b1, W2, b2, W3, b3, n_steps):
    return _build_and_run(
        np.asarray(theta0), np.asarray(context), W1, b1, W2, b2, W3, b3, n_steps
    )
